# revision 2
# baseline (speedup 1.0000x reference)
"""Distributed Trainium2 kernel for nn_AlgebraicLinear (8, 4096, 256) x (256, 256) linear.

out[b, s, o] = sum_i x[b, s, i] * weight[o, i] + bias[o]

Sharding: pure data-parallel — batch dim (8) maps 1:1 onto the 8 NeuronCores.
Per core the GEMM is M=4096 tokens, K=256, N=256.

Layout trick: the host passes x[c].T (256, 4096) and weight.T (256, 256) so the
contraction axis i lands on SBUF partitions with contiguous DMAs (no on-chip
transpose). The device computes out.T tiles (psum [o:128, s:512]) with
weight-stationary float32r (FP22) matmuls; the bias is fused into the
PSUM->SBUF eviction on ScalarE (bias is per-partition in this orientation).
The host transposes the returned out.T back.
"""

import numpy as np

B, S, I, O = 8, 4096, 256, 256
P = 128
SBLK = 512
NS = S // SBLK  # 8
KT = I // P  # 2
OT = O // P  # 2
N_CORES = 8

_CACHE = {}


def _build():
    if "nc" in _CACHE:
        return _CACHE["nc"]

    import concourse.bass as bass  # noqa: F401
    import concourse.mybir as mybir
    from concourse import bacc, tile

    f32 = mybir.dt.float32
    f32r = mybir.dt.float32r

    nc = bacc.Bacc("TRN2", target_bir_lowering=False, debug=False,
                   num_devices=N_CORES)

    xT_ext = nc.dram_tensor("xT", [I, S], f32r, kind="ExternalInput")
    wT_ext = nc.dram_tensor("wT", [I, O], f32r, kind="ExternalInput")
    b_ext = nc.dram_tensor("bias", [O], f32, kind="ExternalInput")
    out_ext = nc.dram_tensor("out", [O, S], f32, kind="ExternalOutput")

    # DRAM access patterns with the k/o tile index split out.
    xT_d = xT_ext.ap().rearrange("(k p) s -> p k s", p=P)      # [128, 2, 4096]
    wT_d = wT_ext.ap().rearrange("(k p) o -> p k o", p=P)      # [128, 2, 256]
    b_d = b_ext.ap().rearrange("(t p) -> p t", p=P)            # [128, 2]
    out_d = out_ext.ap().rearrange("(t p) s -> t p s", p=P)    # [2, 128, 4096]

    with tile.TileContext(nc) as tc:
        with (
            tc.tile_pool(name="const", bufs=1) as const,
            tc.tile_pool(name="xin", bufs=3) as xin,
            tc.tile_pool(name="psum", bufs=4, space="PSUM") as psum_pool,
            tc.tile_pool(name="outp", bufs=4) as outp,
        ):
            w_sb = const.tile([P, KT, O], f32r)
            nc.sync.dma_start(out=w_sb[:], in_=wT_d)
            b_sb = const.tile([P, OT], f32)
            nc.sync.dma_start(out=b_sb[:], in_=b_d)

            for sb in range(NS):
                s0 = sb * SBLK
                x_sb = xin.tile([P, KT, SBLK], f32r)
                nc.sync.dma_start(out=x_sb[:], in_=xT_d[:, :, s0:s0 + SBLK])

                for ot in range(OT):
                    ps = psum_pool.tile([P, SBLK], f32)
                    for k in range(KT):
                        nc.tensor.matmul(
                            ps[:],
                            lhsT=w_sb[:, k, ot * P:(ot + 1) * P],
                            rhs=x_sb[:, k, :],
                            start=(k == 0),
                            stop=(k == KT - 1),
                        )
                    o_sb = outp.tile([P, SBLK], f32)
                    nc.scalar.activation(
                        o_sb[:], ps[:],
                        mybir.ActivationFunctionType.Identity,
                        bias=b_sb[:, ot:ot + 1],
                    )
                    nc.sync.dma_start(
                        out=out_d[ot][:, s0:s0 + SBLK], in_=o_sb[:],
                    )

    nc.compile()
    _CACHE["nc"] = nc
    return nc


def _run(in_maps, trace=False, trace_kwargs=None):
    from concourse.bass_utils import run_bass_kernel_spmd

    nc = _build()
    return run_bass_kernel_spmd(
        nc, in_maps, core_ids=list(range(N_CORES)),
        trace=trace, **(trace_kwargs or {}),
    )


def _make_in_maps(x, weight, bias):
    x = np.asarray(x, dtype=np.float32)
    weight = np.asarray(weight, dtype=np.float32)
    bias = np.asarray(bias, dtype=np.float32)
    wT = np.ascontiguousarray(weight.T)
    in_maps = []
    for c in range(N_CORES):
        in_maps.append({
            "xT": np.ascontiguousarray(x[c].T),
            "wT": wT,
            "bias": bias,
        })
    return in_maps


def kernel(x, weight, bias):
    in_maps = _make_in_maps(x, weight, bias)
    res = _run(in_maps)
    out = np.empty((B, S, O), dtype=np.float32)
    for c in range(N_CORES):
        out[c] = res.results[c]["out"].T
    return out


# revision 3
# speedup vs baseline: 1.3931x; 1.3931x over previous
"""Distributed Trainium2 kernel for nn_AlgebraicLinear (8, 4096, 256) x (256, 256) linear.

out[b, s, o] = sum_i x[b, s, i] * weight[o, i] + bias[o]

Sharding: pure data-parallel — batch dim (8) maps 1:1 onto the 8 NeuronCores.
Per core the GEMM is M=4096 tokens, K=256, N=256.

Layout trick: the host passes x[c].T (256, 4096) and weight.T (256, 256) so the
contraction axis i lands on SBUF partitions with contiguous DMAs (no on-chip
transpose). The device computes out.T tiles (psum [o:128, s:512]) with
weight-stationary float32r (FP22) matmuls; the bias is added during the
PSUM->SBUF eviction on VectorE (per-partition scalar in this orientation).
The host transposes the returned out.T back.

DMA plan: HWDGE issue cost is ~750ns per dma_start per sequencer ring, so
x-loads (4x 1 MiB) ride the Sync ring, out-stores (8x 512 KiB) the Scalar
ring, and the two constant loads go to GpSimd SWDGE.
"""

import numpy as np

B, S, I, O = 8, 4096, 256, 256
P = 128
SBLK = 1024
NS = S // SBLK   # 4
NH = SBLK // 512  # psum-width halves per s-block
KT = I // P      # 2
OT = O // P      # 2
N_CORES = 8

_CACHE = {}


def _build():
    if "nc" in _CACHE:
        return _CACHE["nc"]

    import concourse.bass as bass  # noqa: F401
    import concourse.mybir as mybir
    from concourse import bacc, tile

    f32 = mybir.dt.float32
    f32r = mybir.dt.float32r

    nc = bacc.Bacc("TRN2", target_bir_lowering=False, debug=False,
                   num_devices=N_CORES)

    xT_ext = nc.dram_tensor("xT", [I, S], f32r, kind="ExternalInput")
    wT_ext = nc.dram_tensor("wT", [I, O], f32r, kind="ExternalInput")
    b_ext = nc.dram_tensor("bias", [O], f32, kind="ExternalInput")
    out_ext = nc.dram_tensor("out", [O, S], f32, kind="ExternalOutput")

    # DRAM access patterns with the k/o tile index split out.
    xT_d = xT_ext.ap().rearrange("(k p) s -> p k s", p=P)      # [128, 2, 4096]
    wT_d = wT_ext.ap().rearrange("(k p) o -> p k o", p=P)      # [128, 2, 256]
    b_d = b_ext.ap().rearrange("(t p) -> p t", p=P)            # [128, 2]
    out_d = out_ext.ap().rearrange("(t p) s -> t p s", p=P)    # [2, 128, 4096]

    with tile.TileContext(nc) as tc:
        with (
            tc.tile_pool(name="const", bufs=1) as const,
            tc.tile_pool(name="xin", bufs=NS) as xin,
            tc.tile_pool(name="psum", bufs=8, space="PSUM") as psum_pool,
            tc.tile_pool(name="outp", bufs=4) as outp,
        ):
            w_sb = const.tile([P, KT, O], f32r)
            nc.gpsimd.dma_start(out=w_sb[:], in_=wT_d)
            b_sb = const.tile([P, OT], f32)
            nc.gpsimd.dma_start(out=b_sb[:], in_=b_d)

            for sb in range(NS):
                s0 = sb * SBLK
                x_sb = xin.tile([P, KT, SBLK], f32r)
                nc.sync.dma_start(out=x_sb[:], in_=xT_d[:, :, s0:s0 + SBLK])

                for ot in range(OT):
                    o_sb = outp.tile([P, SBLK], f32)
                    for sh in range(NH):
                        ps = psum_pool.tile([P, 512], f32)
                        for k in range(KT):
                            nc.tensor.matmul(
                                ps[:],
                                lhsT=w_sb[:, k, ot * P:(ot + 1) * P],
                                rhs=x_sb[:, k, sh * 512:(sh + 1) * 512],
                                start=(k == 0),
                                stop=(k == KT - 1),
                            )
                        nc.vector.tensor_scalar_add(
                            o_sb[:, sh * 512:(sh + 1) * 512], ps[:],
                            b_sb[:, ot:ot + 1],
                        )
                    nc.scalar.dma_start(
                        out=out_d[ot][:, s0:s0 + SBLK], in_=o_sb[:],
                    )

    nc.compile()
    _CACHE["nc"] = nc
    return nc


def _run(in_maps, trace=False, trace_kwargs=None):
    from concourse.bass_utils import run_bass_kernel_spmd

    nc = _build()
    return run_bass_kernel_spmd(
        nc, in_maps, core_ids=list(range(N_CORES)),
        trace=trace, **(trace_kwargs or {}),
    )


def _make_in_maps(x, weight, bias):
    x = np.asarray(x, dtype=np.float32)
    weight = np.asarray(weight, dtype=np.float32)
    bias = np.asarray(bias, dtype=np.float32)
    wT = np.ascontiguousarray(weight.T)
    in_maps = []
    for c in range(N_CORES):
        in_maps.append({
            "xT": np.ascontiguousarray(x[c].T),
            "wT": wT,
            "bias": bias,
        })
    return in_maps


def kernel(x, weight, bias):
    in_maps = _make_in_maps(x, weight, bias)
    res = _run(in_maps)
    out = np.empty((B, S, O), dtype=np.float32)
    for c in range(N_CORES):
        out[c] = res.results[c]["out"].T
    return out
